# revision 14
# baseline (speedup 1.0000x reference)
"""Trainium2 Bass kernel for nn_CapsuleLayer (dynamic routing capsule layer).

Math (reference):
    u[n,i,D] = sum_d W[n,i,D,d] * x[i,d]                  (N=64, I=4096, D=32, d=16)
    b = 0
    repeat 3x:
        c = softmax(b, axis=i)
        s[n,D] = sum_i c[n,i] u[n,i,D]
        sq = sum_{n,D} s^2                                 (GLOBAL scalar)
        v = s * sq/(1+sq)/(sqrt(sq)+eps)
        b += sum_D u[n,i,D] v[n,D]
    return v (from last iteration), shape (64, 32, 1)

Sharding: W and u split along n (output capsules) across 8 cores (8 each).
Per routing iteration only the scalar sq needs a cross-core AllReduce.

Per-core pipeline (i = g*128 + p, g in 0..32, p = SBUF partition):

Phase A (memory-bound, target ~180us): stream W with a casting DMA (fp32
HBM read -> fp16 SBUF), then compute u on the TENSOR engine: for each d,
    matmul(psum_u +=, lhsT=diag(x[:,d]) [128x128 fp16], rhs=W16[:, d::16])
i.e. a diagonal stationary operand turns the PE into a per-partition
scalar-broadcast MAC; the d-sum accumulates in PSUM (fp32). The diagonal
weights are built on DVE (ident16 * x, 4x perf mode). u is copied to
SBUF in fp32 (u) and fp16 (u16), and PE-transposed into uT halves
(fp16, partitions (nl*32+D), cols i) for the logit update.

Routing (3 iterations, only a scalar AllReduce crosses cores):
    s_un[n,D] = S0[n,D] + sum_i em1[n,i]*u16[i,(n,D)],  em1 = exp(b)-1
with S0 = sum_i u accumulated in fp32 during phase A (iteration 1 uses
c uniform == S0/4096 exactly). The em1 formulation keeps the fp16
matmul numerically safe: fp16 only carries the O(1e-3) correction.
A ones-column in u16 makes the same matmul emit sum_i em1 (softmax
denominator Z = 4096 + that). The diagonal of the [8,257] PSUM result
is extracted via PE transpose + mask (partition-aligned); the logit
update b += gfac * (u . s) runs as PE matmuls of uT against a
block-diagonal Sdiag (fp16), overlapped with the AllReduce (gfac is a
scalar factor applied afterwards on DVE).
"""

import sys

if "/opt/trn_rl_repo" not in sys.path:
    sys.path.insert(0, "/opt/trn_rl_repo")

import numpy as np

import bass_rust as _bass_rust
import concourse.bass as bass
import concourse.mybir as mybir
import concourse.tile as tile
from concourse.bass_utils import run_bass_kernel_spmd

F32 = mybir.dt.float32
F16 = mybir.dt.float16
BF16 = mybir.dt.bfloat16
ALU = mybir.AluOpType
ACTF = mybir.ActivationFunctionType

N_CORES = 8
N_CAPS = 64
N_LOC = N_CAPS // N_CORES  # 8 output capsules per core
I_CAPS = 4096
CAP_D = 32
IN_D = 16
G = I_CAPS // 128  # 32 i-blocks
NDC = N_LOC * CAP_D  # 256
UBLK = NDC + 1  # 257: u block width incl. ones column
EPS = 1e-7
NUM_ROUTINGS = 3


def _build_nc():
    nc = bass.Bass(trn_type="TRN2", num_devices=N_CORES)

    w = nc.dram_tensor("w", [N_LOC, I_CAPS, CAP_D, IN_D], F32, kind="ExternalInput")
    x = nc.dram_tensor("x", [I_CAPS, IN_D], F32, kind="ExternalInput")
    ident = nc.dram_tensor("ident", [128, 128], F32, kind="ExternalInput")
    ident16 = nc.dram_tensor("ident16", [128, 128], F16, kind="ExternalInput")
    # e_h[n, p] = 1 iff p//32 == n - 4h   (n in 0..8, p in 0..128)
    e0 = nc.dram_tensor("e0", [N_LOC, 128], F32, kind="ExternalInput")
    e1 = nc.dram_tensor("e1", [N_LOC, 128], F32, kind="ExternalInput")
    # mask8_h = e_h.T; mask4[p, nl] = 1 iff p//32 == nl
    m8_0 = nc.dram_tensor("m8_0", [128, N_LOC], F32, kind="ExternalInput")
    m8_1 = nc.dram_tensor("m8_1", [128, N_LOC], F32, kind="ExternalInput")
    m4 = nc.dram_tensor("m4", [128, 4], F32, kind="ExternalInput")
    v_out = nc.dram_tensor("v_out", [N_LOC, CAP_D], F32, kind="ExternalOutput")

    with tile.TileContext(nc) as tc:
        with (
            tc.tile_pool(name="sb", bufs=1) as sb,
            tc.tile_pool(name="sb_w", bufs=3) as wpool,
            tc.tile_pool(name="dram", bufs=1, space="DRAM") as dram,
        ):
            # ---- persistent SBUF tiles ----
            u16 = sb.tile([128, G * UBLK], F16)
            uT0 = sb.tile([128, I_CAPS], F16)
            uT1 = sb.tile([128, I_CAPS], F16)
            uT = (uT0, uT1)
            x_sb = sb.tile([128, G * IN_D], F32)
            ident_sb = sb.tile([128, 128], F32)
            id16_sb = sb.tile([128, 128], F16)
            ones_col = sb.tile([128, 1], F32)
            ones16 = sb.tile([128, 1], F16)
            ones_row = sb.tile([1, 128], F32)
            s0_flat = sb.tile([128, 2], F32)

            nc.sync.dma_start(
                out=x_sb[:].rearrange("p (g d) -> p g d", d=IN_D),
                in_=x.rearrange("(g p) d -> p g d", p=128),
            )
            nc.sync.dma_start(out=ident_sb[:], in_=ident[:])
            nc.sync.dma_start(out=id16_sb[:], in_=ident16[:])
            e_sb = []
            m8_sb = []
            for h, (eh, mh) in enumerate(((e0, m8_0), (e1, m8_1))):
                et = sb.tile([N_LOC, 128], F32, name=f"e{h}_sb", tag=f"e{h}_sb")
                nc.sync.dma_start(out=et[:], in_=eh[:])
                e_sb.append(et)
                mt = sb.tile([128, N_LOC], F32, name=f"m8{h}_sb", tag=f"m8{h}_sb")
                nc.sync.dma_start(out=mt[:], in_=mh[:])
                m8_sb.append(mt)
            m4_sb = sb.tile([128, 4], F32)
            nc.sync.dma_start(out=m4_sb[:], in_=m4[:])
            nc.vector.memset(ones_col[:], 1.0)
            nc.vector.memset(ones16[:], 1.0)
            nc.vector.memset(ones_row[:], 1.0)
            u16_ones = u16[:].rearrange("p (g c) -> p g c", c=UBLK)[
                :, :, UBLK - 1 : UBLK
            ]
            nc.vector.memset(u16_ones, 1.0)

            # Pre-warm the collective path so iteration 1's AllReduce does
            # not pay first-call setup on the critical path (runs on the
            # TOPSP/SDMA engines concurrently with phase A).
            warm_in = dram.tile([1, 8], F32)
            warm_out = dram.tile([1, 8], F32, addr_space="Shared")
            warm_sb = sb.tile([1, 8], F32)
            nc.vector.memset(warm_sb[:], 0.0)
            nc.gpsimd.dma_start(out=warm_in[:], in_=warm_sb[:])
            nc.gpsimd.collective_compute(
                "AllReduce",
                ALU.add,
                replica_groups=[list(range(N_CORES))],
                ins=[warm_in[:].opt()],
                outs=[warm_out[:].opt()],
            )

            # Block-diagonal s tiles (one pair per logit-updating iteration).
            sdiag = {}
            for k in (1, 2):
                for h in (0, 1):
                    t = sb.tile([128, 4], F16, name=f"sd{k}{h}", tag=f"sd{k}{h}")
                    nc.vector.memset(t[:], 0.0)
                    sdiag[(k, h)] = t

            def allreduce_gfac(k, sq_src):
                """AllReduce the partial squash norm and compute the scalar
                factor g = sq/(1+sq)/(sqrt(sq)+eps) as a [1,1] SBUF tile."""
                cc_in = dram.tile([1, 8], F32, name=f"ccin{k}", tag=f"ccin{k}")
                cc_out = dram.tile(
                    [1, 8], F32, name=f"ccout{k}", tag=f"ccout{k}",
                    addr_space="Shared",
                )
                nc.gpsimd.dma_start(out=cc_in[:], in_=sq_src[:])
                nc.gpsimd.collective_compute(
                    "AllReduce",
                    ALU.add,
                    replica_groups=[list(range(N_CORES))],
                    ins=[cc_in[:].opt()],
                    outs=[cc_out[:].opt()],
                )
                sqg = sb.tile([1, 1], F32, name=f"sqg{k}", tag=f"sqg{k}")
                nc.gpsimd.dma_start(out=sqg[:], in_=cc_out[0:1, 0:1])

                # sqrt via exp(0.5*ln(x)): keeps ACT on one table set
                # (natural_log_exp) and is far more accurate than ACT Sqrt.
                lnv = sb.tile([1, 1], F32, name=f"ln{k}", tag=f"ln{k}")
                nc.scalar.activation(lnv[:], sqg[:], ACTF.Ln)
                sqr = sb.tile([1, 1], F32, name=f"sqr{k}", tag=f"sqr{k}")
                nc.scalar.activation(sqr[:], lnv[:], ACTF.Exp, scale=0.5)
                den1 = sb.tile([1, 1], F32, name=f"d1{k}", tag=f"d1{k}")
                nc.vector.tensor_scalar_add(den1[:], sqr[:], EPS)
                den2 = sb.tile([1, 1], F32, name=f"d2{k}", tag=f"d2{k}")
                nc.vector.tensor_scalar_add(den2[:], sqg[:], 1.0)
                den = sb.tile([1, 1], F32, name=f"dn{k}", tag=f"dn{k}")
                nc.vector.tensor_mul(den[:], den1[:], den2[:])
                dinv = sb.tile([1, 1], F32, name=f"di{k}", tag=f"di{k}")
                nc.vector.reciprocal(dinv[:], den[:])
                gf = sb.tile([1, 1], F32, name=f"gf{k}", tag=f"gf{k}")
                nc.vector.tensor_mul(gf[:], sqg[:], dinv[:])
                return gf

            def gfac_bcast(mpool, k, gf, tag="gb"):
                gb = mpool.tile([128, 1], F32, name=f"gb{k}", tag=tag)
                nc.tensor.matmul(
                    gb[:], ones_row[0:1, 0:128], gf[0:1, 0:1], start=True, stop=True
                )
                return gb

            # ================= Phase A: stream W, build u, uT ==============
            with tc.tile_pool(name="ps_s1", bufs=1, space="PSUM") as ps1pool:
                ps_s1 = ps1pool.tile([1, UBLK], F32)
                with (
                    tc.tile_pool(name="ps_u", bufs=2, space="PSUM") as upool,
                    tc.tile_pool(name="ps_tr", bufs=2, space="PSUM") as trpool,
                    tc.tile_pool(name="sb_dg", bufs=32) as dgpool,
                    tc.tile_pool(name="sb_usb", bufs=3) as usbpool,
                ):
                    def emit_u_consumers(g):
                        # transpose u block into uT halves + iteration-1 s
                        # accumulation (ones column -> denominator col 256)
                        for h in (0, 1):
                            tr = trpool.tile([128, 128], F16, name="tr", tag="tr")
                            nc.tensor.transpose(
                                tr[:],
                                u16[:, g * UBLK + h * 128 : g * UBLK + (h + 1) * 128],
                                id16_sb[:],
                            )
                            nc.scalar.copy(
                                uT[h][:, g * 128 : (g + 1) * 128], tr[:]
                            )
                        nc.tensor.matmul(
                            ps_s1[:],
                            ones16[:],
                            u16[:, g * UBLK : (g + 1) * UBLK],
                            start=(g == 0),
                            stop=(g == G - 1),
                        )

                    N_PE = 7   # d-steps on the tensor engine
                    N_ACT = 3  # d-steps as ACT-mult + DVE-add
                    for g in range(G):
                        # casting DMA: reads the full fp32 W from HBM (the
                        # roofline traffic), lands fp16 in SBUF
                        wg = wpool.tile([128, N_LOC * 512], F16, name="wg", tag="wg")
                        nc.gpsimd.dma_start(
                            out=wg[:].rearrange("p (n f) -> p n f", f=512),
                            in_=w[:, g * 128 : (g + 1) * 128, :, :].rearrange(
                                "n p a b -> p n (a b)"
                            ),
                        )

                        def xcol(d):
                            return x_sb[:, g * IN_D + d : g * IN_D + d + 1]

                        # PE part: u_psum = sum_{d<N_PE} diag(x_d) @ W16[:, d::16]
                        # (diagonal stationary operand == per-partition scalar
                        # broadcast MAC; d-sum accumulates in PSUM fp32)
                        up = upool.tile([128, NDC], F32, name="up", tag="up")
                        for d in range(N_PE):
                            dg = dgpool.tile([128, 128], F16, name="dg", tag="dg")
                            nc.scalar.activation(
                                dg[:], id16_sb[:], ACTF.Copy, scale=xcol(d)
                            )
                            nc.tensor.matmul(
                                up[:],
                                dg[:],
                                wg[:, d : 4096 : IN_D],
                                start=(d == 0),
                                stop=(d == N_PE - 1),
                            )
                        # DVE part: fused MACs; ACT part: mults + DVE adds
                        usb = usbpool.tile([128, NDC], F32, name="usb", tag="usb")
                        nc.vector.tensor_scalar_mul(
                            usb[:], wg[:, N_PE : 4096 : IN_D], xcol(N_PE)
                        )
                        for d in range(N_PE + 1, IN_D - N_ACT):
                            nc.vector.scalar_tensor_tensor(
                                usb[:], wg[:, d : 4096 : IN_D], xcol(d), usb[:],
                                ALU.mult, ALU.add,
                            )
                        for d in range(IN_D - N_ACT, IN_D):
                            tmp = usbpool.tile([128, NDC], F32, name="tmp", tag="tmp")
                            nc.scalar.activation(
                                tmp[:], wg[:, d : 4096 : IN_D], ACTF.Copy,
                                scale=xcol(d),
                            )
                            nc.vector.tensor_add(usb[:], usb[:], tmp[:])
                        # u16 = PE part + DVE part (single fp16 u store; PSUM
                        # held the fp32 partials so precision is kept)
                        u_g16 = u16[:, g * UBLK : g * UBLK + NDC]
                        nc.vector.tensor_add(u_g16, usb[:], up[:])
                        # PE consumers of u16 run one block behind (PE's
                        # queue is in-order; emitting them for g-1 keeps PE
                        # from stalling on this block's DVE add)
                        if g > 0:
                            emit_u_consumers(g - 1)
                    emit_u_consumers(G - 1)

                # ========== routing tail (linearized logits) ==========
                # Logits b stay O(1e-3), so exp(b)-1 ~= b to ~1e-6 abs.
                # With b_k = sum_j g_j*DB_j (DB_j = u . s_j), the s-update
                #   s_un_{k+1} = S0 + sum_i (exp(b)-1)*u ~= S0 + sum_j g_j*M_j
                # where M_j = sum_i DB_j[i,n]*u16[i,(n,D)] does NOT depend on
                # the AllReduced scalar g_j -- so the heavy DB/M matmuls run
                # DURING each AllReduce; only tiny [8,257] ops remain on the
                # serial path after it.
                with tc.tile_pool(name="ps_s0t", bufs=1, space="PSUM") as s0pool:
                    r1 = sb.tile([1, 1], F32)
                    nc.vector.reciprocal(r1[:], ps_s1[0:1, UBLK - 1 : UBLK])
                    s0_row = sb.tile([1, NDC], F32)
                    nc.scalar.copy(s0_row[:], ps_s1[0:1, 0:NDC])
                    for h in (0, 1):
                        s0t = s0pool.tile([128, 1], F32, name=f"s0t{h}", tag=f"s0t{h}")
                        nc.tensor.transpose(
                            s0t[:],
                            s0_row[0:1, h * 128 : (h + 1) * 128],
                            ident_sb[0:1, 0:1],
                        )
                        nc.vector.tensor_copy(s0_flat[:, h : h + 1], s0t[:])
                    s_row = sb.tile([1, NDC], F32)
                    nc.vector.tensor_scalar_mul(s_row[:], ps_s1[0:1, 0:NDC], r1[:])
                    junk_row = sb.tile([1, NDC], F32)
                    sq1 = sb.tile([1, 8], F32)
                    nc.vector.memset(sq1[:], 0.0)
                    nc.vector.scalar_tensor_tensor(
                        junk_row[:], s_row[:], 1.0, s_row[:],
                        ALU.mult, ALU.mult, accum_out=sq1[0:1, 0:1],
                    )

            with (
                tc.tile_pool(name="ps_db", bufs=1, space="PSUM") as dbpool,
                tc.tile_pool(name="ps_M", bufs=1, space="PSUM") as Mpool,
                tc.tile_pool(name="ps_T", bufs=2, space="PSUM") as tpool,
                tc.tile_pool(name="ps_rb", bufs=1, space="PSUM") as rpool,
                tc.tile_pool(name="ps_m2", bufs=2, space="PSUM") as mpool,
            ):
                C = sb.tile([N_LOC, UBLK], F32)

                def db_and_M(k):
                    """DB_k = u . s_k (via uT x Sdiag), then M_k = DB_k^T-ish
                    contraction with u16 -- all independent of g_k, so this
                    overlaps AllReduce k."""
                    db16 = sb.tile(
                        [128, G * N_LOC], F16, name=f"db16_{k}", tag="db16"
                    )
                    for h in (0, 1):
                        dbp = [
                            dbpool.tile(
                                [128, 64], F32, name=f"db{k}{h}{p}", tag=f"db{p}"
                            )
                            for p in (0, 1)
                        ]
                        for c in range(G):
                            nc.tensor.matmul(
                                dbp[c % 2][:, (c // 2) * 4 : (c // 2 + 1) * 4],
                                uT[h][:, c * 128 : (c + 1) * 128],
                                sdiag[(k, h)][:, 0:4],
                                start=True,
                                stop=True,
                            )
                        for p in (0, 1):
                            o_v = db16[:].rearrange("p (g n) -> p g n", n=N_LOC)[
                                :, p::2, h * 4 : (h + 1) * 4
                            ]
                            nc.scalar.copy(
                                o_v, dbp[p][:].rearrange("p (c n) -> p c n", n=4)
                            )
                    ps_M = Mpool.tile([N_LOC, UBLK], F32, name=f"psM{k}", tag="psM")
                    for g in range(G):
                        nc.tensor.matmul(
                            ps_M[:],
                            db16[:, g * N_LOC : (g + 1) * N_LOC],
                            u16[:, g * UBLK : (g + 1) * UBLK],
                            start=(g == 0),
                            stop=(g == G - 1),
                        )
                    return ps_M

                def extract(k):
                    """C (+S0) -> s_un/s_true in partition-flat layout, plus
                    the squash-norm partial; returns (s_un, s_true, rbc, sqk)."""
                    zs = sb.tile([N_LOC, 1], F32, name=f"zs{k}", tag=f"zs{k}")
                    nc.vector.tensor_scalar_add(
                        zs[:], C[:, UBLK - 1 : UBLK], float(I_CAPS)
                    )
                    r8 = sb.tile([N_LOC, 1], F32, name=f"r8_{k}", tag=f"r8_{k}")
                    nc.vector.reciprocal(r8[:], zs[:])
                    s_true = sb.tile([128, 2], F32, name=f"st{k}", tag=f"st{k}")
                    s_un = sb.tile([128, 2], F32, name=f"sun{k}", tag=f"sun{k}")
                    rb = rpool.tile([128, 2], F32, name=f"rb{k}", tag="rb")
                    for h in (0, 1):
                        T_h = tpool.tile([128, N_LOC], F32, name=f"T{k}{h}", tag="T")
                        nc.tensor.transpose(
                            T_h[:],
                            C[0:N_LOC, h * 128 : (h + 1) * 128],
                            ident_sb[0:N_LOC, 0:N_LOC],
                        )
                        nc.tensor.matmul(
                            rb[:, h : h + 1], e_sb[h][:, :], r8[:, 0:1],
                            start=True, stop=True,
                        )
                        tmp = sb.tile([128, N_LOC], F32, name=f"tm{k}{h}", tag="tm")
                        nc.vector.tensor_mul(tmp[:], T_h[:], m8_sb[h][:])
                        sc = sb.tile([128, 1], F32, name=f"sc{k}{h}", tag="sc")
                        nc.vector.reduce_sum(sc[:], tmp[:], axis=mybir.AxisListType.X)
                        nc.vector.tensor_add(
                            s_un[:, h : h + 1], sc[:], s0_flat[:, h : h + 1]
                        )
                        nc.vector.tensor_scalar_mul(
                            s_true[:, h : h + 1], s_un[:, h : h + 1], rb[:, h : h + 1]
                        )
                    s_sq = sb.tile([128, 2], F32, name=f"ssq{k}", tag=f"ssq{k}")
                    nc.vector.tensor_mul(s_sq[:], s_true[:], s_true[:])
                    ps_sq = Mpool.tile([1, 2], F32, name=f"pssq{k}", tag="psM")
                    nc.tensor.matmul(
                        ps_sq[:], ones_col[:], s_sq[:], start=True, stop=True
                    )
                    sqk = sb.tile([1, 8], F32, name=f"sqk{k}", tag=f"sqk{k}")
                    nc.vector.memset(sqk[:], 0.0)
                    nc.vector.reduce_sum(
                        sqk[0:1, 0:1], ps_sq[0:1, 0:2], axis=mybir.AxisListType.X
                    )
                    return s_un, s_true, rb, sqk

                def g8_of(k, gf):
                    g8 = mpool.tile([N_LOC, 1], F32, name=f"g8_{k}", tag="m2")
                    nc.tensor.matmul(
                        g8[:], ones_row[0:1, 0:N_LOC], gf[0:1, 0:1],
                        start=True, stop=True,
                    )
                    return g8

                # -- iteration 1: sq1 -> AR1; DB1/M1 overlap the AllReduce --
                gf1 = allreduce_gfac(1, sq1)
                for n_ in range(N_LOC):
                    h, nl = n_ // 4, n_ % 4
                    nc.gpsimd.dma_start(
                        out=sdiag[(1, h)][nl * 32 : (nl + 1) * 32, nl : nl + 1],
                        in_=s_row[0:1, n_ * 32 : (n_ + 1) * 32],
                    )
                ps_M1 = db_and_M(1)
                g81 = g8_of(1, gf1)
                nc.vector.tensor_scalar_mul(C[:], ps_M1[:], g81[:, 0:1])

                # -- iteration 2 --
                s_un2, s_true2, rb2, sq2 = extract(2)
                gf2 = allreduce_gfac(2, sq2)
                for h in (0, 1):
                    nc.vector.tensor_scalar(
                        sdiag[(2, h)][:], m4_sb[:], s_un2[:, h : h + 1],
                        rb2[:, h : h + 1], ALU.mult, ALU.mult,
                    )
                ps_M2 = db_and_M(2)
                g82 = g8_of(2, gf2)
                nc.vector.scalar_tensor_tensor(
                    C[:], ps_M2[:], g82[:, 0:1], C[:], ALU.mult, ALU.add
                )

                # -- iteration 3 --
                s_un3, s_true3, rb3, sq3 = extract(3)
                gf3 = allreduce_gfac(3, sq3)
                gb3 = gfac_bcast(mpool, 3, gf3, tag="m2")
                v_flat = sb.tile([128, 2], F32)
                nc.vector.tensor_scalar_mul(v_flat[:], s_true3[:], gb3[:, 0:1])
                for h in (0, 1):
                    nc.sync.dma_start(
                        out=v_out[h * 4 : (h + 1) * 4, :],
                        in_=v_flat[:, h : h + 1],
                    )

    # The SPMD/axon path serializes nc.m directly without running Bacc's
    # pass pipeline; this walrus build allows at most one sync wait per
    # instruction, so split multi-waits into EventSemaphore instructions.
    _bass_rust.generate_event_semaphores(nc)
    return nc


_NC_CACHE = None


def _get_nc():
    global _NC_CACHE
    if _NC_CACHE is None:
        _NC_CACHE = _build_nc()
    return _NC_CACHE


def kernel(input_data, W, _trace=False, _tmpdir=None):
    input_data = np.ascontiguousarray(np.asarray(input_data, dtype=np.float32))
    W = np.ascontiguousarray(np.asarray(W, dtype=np.float32))
    assert input_data.shape == (I_CAPS, IN_D, 1)
    assert W.shape == (N_CAPS, I_CAPS, CAP_D, IN_D)

    x2 = np.ascontiguousarray(input_data[:, :, 0])
    eye = np.eye(128, dtype=np.float32)
    p_grp = np.arange(128) // 32  # partition -> local capsule index
    e_h = []
    for h in (0, 1):
        e = np.zeros((N_LOC, 128), dtype=np.float32)
        for n_ in range(N_LOC):
            e[n_] = (p_grp == n_ - 4 * h).astype(np.float32)
        e_h.append(e)
    m4_np = (p_grp[:, None] == np.arange(4)[None, :]).astype(np.float32)
    consts = {
        "ident": eye,
        "ident16": eye.astype(np.float16),
        "e0": e_h[0],
        "e1": e_h[1],
        "m8_0": np.ascontiguousarray(e_h[0].T),
        "m8_1": np.ascontiguousarray(e_h[1].T),
        "m4": m4_np,
    }
    in_maps = [
        {
            "w": np.ascontiguousarray(W[c * N_LOC : (c + 1) * N_LOC]),
            "x": x2,
            **consts,
        }
        for c in range(N_CORES)
    ]
    nc = _get_nc()
    out = run_bass_kernel_spmd(
        nc,
        in_maps,
        core_ids=list(range(N_CORES)),
        trace=_trace,
        tmpdir=_tmpdir,
    )
    res = out.results if hasattr(out, "results") else out
    v = np.concatenate([res[c]["v_out"] for c in range(N_CORES)], axis=0)
    if _trace:
        kernel.last_exec_time_ns = out.exec_time_ns
        kernel.last_results = out
    return v[..., None].astype(np.float32)


if __name__ == "__main__":
    rng = np.random.default_rng(0)
    inp = {
        "input_data": rng.standard_normal((I_CAPS, IN_D, 1)).astype(np.float32),
        "W": (rng.standard_normal((N_CAPS, I_CAPS, CAP_D, IN_D)) * 0.05).astype(
            np.float32
        ),
    }
    v = kernel(**inp)
    print("kernel output:", v.shape, v.dtype, "norm", np.linalg.norm(v))


# revision 15
# speedup vs baseline: 1.0592x; 1.0592x over previous
"""Trainium2 Bass kernel for nn_CapsuleLayer (dynamic routing capsule layer).

Math (reference):
    u[n,i,D] = sum_d W[n,i,D,d] * x[i,d]                  (N=64, I=4096, D=32, d=16)
    b = 0
    repeat 3x:
        c = softmax(b, axis=i)
        s[n,D] = sum_i c[n,i] u[n,i,D]
        sq = sum_{n,D} s^2                                 (GLOBAL scalar)
        v = s * sq/(1+sq)/(sqrt(sq)+eps)
        b += sum_D u[n,i,D] v[n,D]
    return v (from last iteration), shape (64, 32, 1)

Sharding: W and u split along n (output capsules) across 8 cores (8 each).
Per routing iteration only the scalar sq needs a cross-core AllReduce.

Per-core pipeline (i = g*128 + p, g in 0..32, p = SBUF partition):

Phase A (memory-bound, target ~180us): stream W with a casting DMA (fp32
HBM read -> fp16 SBUF), then compute u on the TENSOR engine: for each d,
    matmul(psum_u +=, lhsT=diag(x[:,d]) [128x128 fp16], rhs=W16[:, d::16])
i.e. a diagonal stationary operand turns the PE into a per-partition
scalar-broadcast MAC; the d-sum accumulates in PSUM (fp32). The diagonal
weights are built on DVE (ident16 * x, 4x perf mode). u is copied to
SBUF in fp32 (u) and fp16 (u16), and PE-transposed into uT halves
(fp16, partitions (nl*32+D), cols i) for the logit update.

Routing (3 iterations, only a scalar AllReduce crosses cores):
    s_un[n,D] = S0[n,D] + sum_i em1[n,i]*u16[i,(n,D)],  em1 = exp(b)-1
with S0 = sum_i u accumulated in fp32 during phase A (iteration 1 uses
c uniform == S0/4096 exactly). The em1 formulation keeps the fp16
matmul numerically safe: fp16 only carries the O(1e-3) correction.
A ones-column in u16 makes the same matmul emit sum_i em1 (softmax
denominator Z = 4096 + that). The diagonal of the [8,257] PSUM result
is extracted via PE transpose + mask (partition-aligned); the logit
update b += gfac * (u . s) runs as PE matmuls of uT against a
block-diagonal Sdiag (fp16), overlapped with the AllReduce (gfac is a
scalar factor applied afterwards on DVE).
"""

import sys

if "/opt/trn_rl_repo" not in sys.path:
    sys.path.insert(0, "/opt/trn_rl_repo")

import numpy as np

import bass_rust as _bass_rust
import concourse.bass as bass
import concourse.mybir as mybir
import concourse.tile as tile
from concourse.bass_utils import run_bass_kernel_spmd

F32 = mybir.dt.float32
F16 = mybir.dt.float16
BF16 = mybir.dt.bfloat16
ALU = mybir.AluOpType
ACTF = mybir.ActivationFunctionType

N_CORES = 8
N_CAPS = 64
N_LOC = N_CAPS // N_CORES  # 8 output capsules per core
I_CAPS = 4096
CAP_D = 32
IN_D = 16
G = I_CAPS // 128  # 32 i-blocks
NDC = N_LOC * CAP_D  # 256
UBLK = NDC + 1  # 257: u block width incl. ones column
EPS = 1e-7
NUM_ROUTINGS = 3


def _build_nc():
    nc = bass.Bass(trn_type="TRN2", num_devices=N_CORES)

    w = nc.dram_tensor("w", [N_LOC, I_CAPS, CAP_D, IN_D], F32, kind="ExternalInput")
    x = nc.dram_tensor("x", [I_CAPS, IN_D], F32, kind="ExternalInput")
    ident = nc.dram_tensor("ident", [128, 128], F32, kind="ExternalInput")
    ident16 = nc.dram_tensor("ident16", [128, 128], F16, kind="ExternalInput")
    # e_h[n, p] = 1 iff p//32 == n - 4h   (n in 0..8, p in 0..128)
    e0 = nc.dram_tensor("e0", [N_LOC, 128], F32, kind="ExternalInput")
    e1 = nc.dram_tensor("e1", [N_LOC, 128], F32, kind="ExternalInput")
    # mask8_h = e_h.T; mask4[p, nl] = 1 iff p//32 == nl
    m8_0 = nc.dram_tensor("m8_0", [128, N_LOC], F32, kind="ExternalInput")
    m8_1 = nc.dram_tensor("m8_1", [128, N_LOC], F32, kind="ExternalInput")
    m4 = nc.dram_tensor("m4", [128, 4], F32, kind="ExternalInput")
    v_out = nc.dram_tensor("v_out", [N_LOC, CAP_D], F32, kind="ExternalOutput")

    with tile.TileContext(nc) as tc:
        with (
            tc.tile_pool(name="sb", bufs=1) as sb,
            tc.tile_pool(name="sb_w", bufs=3) as wpool,
            tc.tile_pool(name="dram", bufs=1, space="DRAM") as dram,
        ):
            # ---- persistent SBUF tiles ----
            u16 = sb.tile([128, G * UBLK], F16)
            uT0 = sb.tile([128, I_CAPS], F16)
            uT1 = sb.tile([128, I_CAPS], F16)
            uT = (uT0, uT1)
            x_sb = sb.tile([128, G * IN_D], F32)
            ident_sb = sb.tile([128, 128], F32)
            id16_sb = sb.tile([128, 128], F16)
            ones_col = sb.tile([128, 1], F32)
            ones16 = sb.tile([128, 1], F16)
            ones_row = sb.tile([1, 128], F32)
            s0_flat = sb.tile([128, 2], F32)

            nc.sync.dma_start(
                out=x_sb[:].rearrange("p (g d) -> p g d", d=IN_D),
                in_=x.rearrange("(g p) d -> p g d", p=128),
            )
            nc.sync.dma_start(out=ident_sb[:], in_=ident[:])
            nc.sync.dma_start(out=id16_sb[:], in_=ident16[:])
            e_sb = []
            m8_sb = []
            for h, (eh, mh) in enumerate(((e0, m8_0), (e1, m8_1))):
                et = sb.tile([N_LOC, 128], F32, name=f"e{h}_sb", tag=f"e{h}_sb")
                nc.sync.dma_start(out=et[:], in_=eh[:])
                e_sb.append(et)
                mt = sb.tile([128, N_LOC], F32, name=f"m8{h}_sb", tag=f"m8{h}_sb")
                nc.sync.dma_start(out=mt[:], in_=mh[:])
                m8_sb.append(mt)
            m4_sb = sb.tile([128, 4], F32)
            nc.sync.dma_start(out=m4_sb[:], in_=m4[:])
            nc.vector.memset(ones_col[:], 1.0)
            nc.vector.memset(ones16[:], 1.0)
            nc.vector.memset(ones_row[:], 1.0)
            u16_ones = u16[:].rearrange("p (g c) -> p g c", c=UBLK)[
                :, :, UBLK - 1 : UBLK
            ]
            nc.vector.memset(u16_ones, 1.0)

            # Pre-warm the collective path so iteration 1's AllReduce does
            # not pay first-call setup on the critical path (runs on the
            # TOPSP/SDMA engines concurrently with phase A).
            warm_in = dram.tile([1, 8], F32)
            warm_out = dram.tile([1, 8], F32, addr_space="Shared")
            warm_sb = sb.tile([1, 8], F32)
            nc.vector.memset(warm_sb[:], 0.0)
            nc.gpsimd.dma_start(out=warm_in[:], in_=warm_sb[:])
            nc.gpsimd.collective_compute(
                "AllReduce",
                ALU.add,
                replica_groups=[list(range(N_CORES))],
                ins=[warm_in[:].opt()],
                outs=[warm_out[:].opt()],
            )

            # Block-diagonal s tiles (one pair per logit-updating iteration).
            sdiag = {}
            for k in (1, 2):
                for h in (0, 1):
                    t = sb.tile([128, 4], F16, name=f"sd{k}{h}", tag=f"sd{k}{h}")
                    nc.vector.memset(t[:], 0.0)
                    sdiag[(k, h)] = t

            def allreduce_gfac(k, sq_src):
                """AllReduce the partial squash norm and compute the scalar
                factor g = sq/(1+sq)/(sqrt(sq)+eps) as a [1,1] SBUF tile."""
                cc_in = dram.tile([1, 8], F32, name=f"ccin{k}", tag=f"ccin{k}")
                cc_out = dram.tile(
                    [1, 8], F32, name=f"ccout{k}", tag=f"ccout{k}",
                    addr_space="Shared",
                )
                nc.gpsimd.dma_start(out=cc_in[:], in_=sq_src[:])
                nc.gpsimd.collective_compute(
                    "AllReduce",
                    ALU.add,
                    replica_groups=[list(range(N_CORES))],
                    ins=[cc_in[:].opt()],
                    outs=[cc_out[:].opt()],
                )
                sqg = sb.tile([1, 1], F32, name=f"sqg{k}", tag=f"sqg{k}")
                nc.gpsimd.dma_start(out=sqg[:], in_=cc_out[0:1, 0:1])

                # sqrt via exp(0.5*ln(x)): keeps ACT on one table set
                # (natural_log_exp) and is far more accurate than ACT Sqrt.
                lnv = sb.tile([1, 1], F32, name=f"ln{k}", tag=f"ln{k}")
                nc.scalar.activation(lnv[:], sqg[:], ACTF.Ln)
                sqr = sb.tile([1, 1], F32, name=f"sqr{k}", tag=f"sqr{k}")
                nc.scalar.activation(sqr[:], lnv[:], ACTF.Exp, scale=0.5)
                den1 = sb.tile([1, 1], F32, name=f"d1{k}", tag=f"d1{k}")
                nc.vector.tensor_scalar_add(den1[:], sqr[:], EPS)
                den2 = sb.tile([1, 1], F32, name=f"d2{k}", tag=f"d2{k}")
                nc.vector.tensor_scalar_add(den2[:], sqg[:], 1.0)
                den = sb.tile([1, 1], F32, name=f"dn{k}", tag=f"dn{k}")
                nc.vector.tensor_mul(den[:], den1[:], den2[:])
                dinv = sb.tile([1, 1], F32, name=f"di{k}", tag=f"di{k}")
                nc.vector.reciprocal(dinv[:], den[:])
                gf = sb.tile([1, 1], F32, name=f"gf{k}", tag=f"gf{k}")
                nc.vector.tensor_mul(gf[:], sqg[:], dinv[:])
                return gf

            def gfac_bcast(mpool, k, gf, tag="gb"):
                gb = mpool.tile([128, 1], F32, name=f"gb{k}", tag=tag)
                nc.tensor.matmul(
                    gb[:], ones_row[0:1, 0:128], gf[0:1, 0:1], start=True, stop=True
                )
                return gb

            # ================= Phase A: stream W, build u, uT ==============
            with tc.tile_pool(name="ps_s1", bufs=1, space="PSUM") as ps1pool:
                ps_s1 = ps1pool.tile([1, UBLK], F32)
                with (
                    tc.tile_pool(name="ps_u", bufs=2, space="PSUM") as upool,
                    tc.tile_pool(name="ps_tr", bufs=2, space="PSUM") as trpool,
                    tc.tile_pool(name="sb_dg", bufs=32) as dgpool,
                    tc.tile_pool(name="sb_usb", bufs=3) as usbpool,
                ):
                    def emit_u_consumers(g):
                        # transpose u block into uT halves + iteration-1 s
                        # accumulation (ones column -> denominator col 256)
                        for h in (0, 1):
                            tr = trpool.tile([128, 128], F16, name="tr", tag="tr")
                            nc.tensor.transpose(
                                tr[:],
                                u16[:, g * UBLK + h * 128 : g * UBLK + (h + 1) * 128],
                                id16_sb[:],
                            )
                            nc.scalar.copy(
                                uT[h][:, g * 128 : (g + 1) * 128], tr[:]
                            )
                        nc.tensor.matmul(
                            ps_s1[:],
                            ones16[:],
                            u16[:, g * UBLK : (g + 1) * UBLK],
                            start=(g == 0),
                            stop=(g == G - 1),
                        )

                    N_PE = 8   # d-steps on the tensor engine
                    N_ACT = 2  # d-steps as ACT-mult + DVE-add
                    for g in range(G):
                        # casting DMA: reads the full fp32 W from HBM (the
                        # roofline traffic), lands fp16 in SBUF
                        wg = wpool.tile([128, N_LOC * 512], F16, name="wg", tag="wg")
                        nc.gpsimd.dma_start(
                            out=wg[:].rearrange("p (n f) -> p n f", f=512),
                            in_=w[:, g * 128 : (g + 1) * 128, :, :].rearrange(
                                "n p a b -> p n (a b)"
                            ),
                        )

                        def xcol(d):
                            return x_sb[:, g * IN_D + d : g * IN_D + d + 1]

                        # PE part: u_psum = sum_{d<N_PE} diag(x_d) @ W16[:, d::16]
                        # (diagonal stationary operand == per-partition scalar
                        # broadcast MAC; d-sum accumulates in PSUM fp32)
                        up = upool.tile([128, NDC], F32, name="up", tag="up")
                        for d in range(N_PE):
                            dg = dgpool.tile([128, 128], F16, name="dg", tag="dg")
                            nc.scalar.activation(
                                dg[:], id16_sb[:], ACTF.Copy, scale=xcol(d)
                            )
                            nc.tensor.matmul(
                                up[:],
                                dg[:],
                                wg[:, d : 4096 : IN_D],
                                start=(d == 0),
                                stop=(d == N_PE - 1),
                            )
                        # DVE part: fused MACs; ACT part: mults + DVE adds
                        usb = usbpool.tile([128, NDC], F32, name="usb", tag="usb")
                        nc.vector.tensor_scalar_mul(
                            usb[:], wg[:, N_PE : 4096 : IN_D], xcol(N_PE)
                        )
                        for d in range(N_PE + 1, IN_D - N_ACT):
                            nc.vector.scalar_tensor_tensor(
                                usb[:], wg[:, d : 4096 : IN_D], xcol(d), usb[:],
                                ALU.mult, ALU.add,
                            )
                        for d in range(IN_D - N_ACT, IN_D):
                            tmp = usbpool.tile([128, NDC], F32, name="tmp", tag="tmp")
                            nc.scalar.activation(
                                tmp[:], wg[:, d : 4096 : IN_D], ACTF.Copy,
                                scale=xcol(d),
                            )
                            nc.vector.tensor_add(usb[:], usb[:], tmp[:])
                        # u16 = PE part + DVE part (single fp16 u store; PSUM
                        # held the fp32 partials so precision is kept)
                        u_g16 = u16[:, g * UBLK : g * UBLK + NDC]
                        nc.vector.tensor_add(u_g16, usb[:], up[:])
                        # PE consumers of u16 run one block behind (PE's
                        # queue is in-order; emitting them for g-1 keeps PE
                        # from stalling on this block's DVE add)
                        if g > 0:
                            emit_u_consumers(g - 1)
                    emit_u_consumers(G - 1)

                # ========== routing tail (linearized logits) ==========
                # Logits b stay O(1e-3), so exp(b)-1 ~= b to ~1e-6 abs.
                # With b_k = sum_j g_j*DB_j (DB_j = u . s_j), the s-update
                #   s_un_{k+1} = S0 + sum_i (exp(b)-1)*u ~= S0 + sum_j g_j*M_j
                # where M_j = sum_i DB_j[i,n]*u16[i,(n,D)] does NOT depend on
                # the AllReduced scalar g_j -- so the heavy DB/M matmuls run
                # DURING each AllReduce; only tiny [8,257] ops remain on the
                # serial path after it.
                with tc.tile_pool(name="ps_s0t", bufs=1, space="PSUM") as s0pool:
                    r1 = sb.tile([1, 1], F32)
                    nc.vector.reciprocal(r1[:], ps_s1[0:1, UBLK - 1 : UBLK])
                    s0_row = sb.tile([1, NDC], F32)
                    nc.scalar.copy(s0_row[:], ps_s1[0:1, 0:NDC])
                    for h in (0, 1):
                        s0t = s0pool.tile([128, 1], F32, name=f"s0t{h}", tag=f"s0t{h}")
                        nc.tensor.transpose(
                            s0t[:],
                            s0_row[0:1, h * 128 : (h + 1) * 128],
                            ident_sb[0:1, 0:1],
                        )
                        nc.vector.tensor_copy(s0_flat[:, h : h + 1], s0t[:])
                    s_row = sb.tile([1, NDC], F32)
                    nc.vector.tensor_scalar_mul(s_row[:], ps_s1[0:1, 0:NDC], r1[:])
                    junk_row = sb.tile([1, NDC], F32)
                    sq1 = sb.tile([1, 8], F32)
                    nc.vector.memset(sq1[:], 0.0)
                    nc.vector.scalar_tensor_tensor(
                        junk_row[:], s_row[:], 1.0, s_row[:],
                        ALU.mult, ALU.mult, accum_out=sq1[0:1, 0:1],
                    )

            with (
                tc.tile_pool(name="ps_db", bufs=1, space="PSUM") as dbpool,
                tc.tile_pool(name="ps_M", bufs=1, space="PSUM") as Mpool,
                tc.tile_pool(name="ps_T", bufs=2, space="PSUM") as tpool,
                tc.tile_pool(name="ps_rb", bufs=1, space="PSUM") as rpool,
                tc.tile_pool(name="ps_m2", bufs=2, space="PSUM") as mpool,
            ):
                C = sb.tile([N_LOC, UBLK], F32)

                def db_and_M(k):
                    """DB_k = u . s_k (via uT x Sdiag), then M_k = DB_k^T-ish
                    contraction with u16 -- all independent of g_k, so this
                    overlaps AllReduce k."""
                    db16 = sb.tile(
                        [128, G * N_LOC], F16, name=f"db16_{k}", tag="db16"
                    )
                    for h in (0, 1):
                        dbp = [
                            dbpool.tile(
                                [128, 64], F32, name=f"db{k}{h}{p}", tag=f"db{p}"
                            )
                            for p in (0, 1)
                        ]
                        for c in range(G):
                            nc.tensor.matmul(
                                dbp[c % 2][:, (c // 2) * 4 : (c // 2 + 1) * 4],
                                uT[h][:, c * 128 : (c + 1) * 128],
                                sdiag[(k, h)][:, 0:4],
                                start=True,
                                stop=True,
                            )
                        for p in (0, 1):
                            o_v = db16[:].rearrange("p (g n) -> p g n", n=N_LOC)[
                                :, p::2, h * 4 : (h + 1) * 4
                            ]
                            nc.scalar.copy(
                                o_v, dbp[p][:].rearrange("p (c n) -> p c n", n=4)
                            )
                    ps_M = Mpool.tile([N_LOC, UBLK], F32, name=f"psM{k}", tag="psM")
                    for g in range(G):
                        nc.tensor.matmul(
                            ps_M[:],
                            db16[:, g * N_LOC : (g + 1) * N_LOC],
                            u16[:, g * UBLK : (g + 1) * UBLK],
                            start=(g == 0),
                            stop=(g == G - 1),
                        )
                    return ps_M

                def extract(k):
                    """C (+S0) -> s_un/s_true in partition-flat layout, plus
                    the squash-norm partial; returns (s_un, s_true, rbc, sqk)."""
                    zs = sb.tile([N_LOC, 1], F32, name=f"zs{k}", tag=f"zs{k}")
                    nc.vector.tensor_scalar_add(
                        zs[:], C[:, UBLK - 1 : UBLK], float(I_CAPS)
                    )
                    r8 = sb.tile([N_LOC, 1], F32, name=f"r8_{k}", tag=f"r8_{k}")
                    nc.vector.reciprocal(r8[:], zs[:])
                    s_true = sb.tile([128, 2], F32, name=f"st{k}", tag=f"st{k}")
                    s_un = sb.tile([128, 2], F32, name=f"sun{k}", tag=f"sun{k}")
                    rb = rpool.tile([128, 2], F32, name=f"rb{k}", tag="rb")
                    for h in (0, 1):
                        T_h = tpool.tile([128, N_LOC], F32, name=f"T{k}{h}", tag="T")
                        nc.tensor.transpose(
                            T_h[:],
                            C[0:N_LOC, h * 128 : (h + 1) * 128],
                            ident_sb[0:N_LOC, 0:N_LOC],
                        )
                        nc.tensor.matmul(
                            rb[:, h : h + 1], e_sb[h][:, :], r8[:, 0:1],
                            start=True, stop=True,
                        )
                        tmp = sb.tile([128, N_LOC], F32, name=f"tm{k}{h}", tag="tm")
                        nc.vector.tensor_mul(tmp[:], T_h[:], m8_sb[h][:])
                        sc = sb.tile([128, 1], F32, name=f"sc{k}{h}", tag="sc")
                        nc.vector.reduce_sum(sc[:], tmp[:], axis=mybir.AxisListType.X)
                        nc.vector.tensor_add(
                            s_un[:, h : h + 1], sc[:], s0_flat[:, h : h + 1]
                        )
                        nc.vector.tensor_scalar_mul(
                            s_true[:, h : h + 1], s_un[:, h : h + 1], rb[:, h : h + 1]
                        )
                    s_sq = sb.tile([128, 2], F32, name=f"ssq{k}", tag=f"ssq{k}")
                    nc.vector.tensor_mul(s_sq[:], s_true[:], s_true[:])
                    ps_sq = Mpool.tile([1, 2], F32, name=f"pssq{k}", tag="psM")
                    nc.tensor.matmul(
                        ps_sq[:], ones_col[:], s_sq[:], start=True, stop=True
                    )
                    sqk = sb.tile([1, 8], F32, name=f"sqk{k}", tag=f"sqk{k}")
                    nc.vector.memset(sqk[:], 0.0)
                    nc.vector.reduce_sum(
                        sqk[0:1, 0:1], ps_sq[0:1, 0:2], axis=mybir.AxisListType.X
                    )
                    return s_un, s_true, rb, sqk

                def g8_of(k, gf):
                    g8 = mpool.tile([N_LOC, 1], F32, name=f"g8_{k}", tag="m2")
                    nc.tensor.matmul(
                        g8[:], ones_row[0:1, 0:N_LOC], gf[0:1, 0:1],
                        start=True, stop=True,
                    )
                    return g8

                # -- iteration 1: sq1 -> AR1; DB1/M1 overlap the AllReduce --
                gf1 = allreduce_gfac(1, sq1)
                for n_ in range(N_LOC):
                    h, nl = n_ // 4, n_ % 4
                    nc.gpsimd.dma_start(
                        out=sdiag[(1, h)][nl * 32 : (nl + 1) * 32, nl : nl + 1],
                        in_=s_row[0:1, n_ * 32 : (n_ + 1) * 32],
                    )
                ps_M1 = db_and_M(1)
                g81 = g8_of(1, gf1)
                nc.vector.tensor_scalar_mul(C[:], ps_M1[:], g81[:, 0:1])

                # -- iteration 2 --
                s_un2, s_true2, rb2, sq2 = extract(2)
                gf2 = allreduce_gfac(2, sq2)
                for h in (0, 1):
                    nc.vector.tensor_scalar(
                        sdiag[(2, h)][:], m4_sb[:], s_un2[:, h : h + 1],
                        rb2[:, h : h + 1], ALU.mult, ALU.mult,
                    )
                ps_M2 = db_and_M(2)
                g82 = g8_of(2, gf2)
                nc.vector.scalar_tensor_tensor(
                    C[:], ps_M2[:], g82[:, 0:1], C[:], ALU.mult, ALU.add
                )

                # -- iteration 3 --
                s_un3, s_true3, rb3, sq3 = extract(3)
                gf3 = allreduce_gfac(3, sq3)
                gb3 = gfac_bcast(mpool, 3, gf3, tag="m2")
                v_flat = sb.tile([128, 2], F32)
                nc.vector.tensor_scalar_mul(v_flat[:], s_true3[:], gb3[:, 0:1])
                for h in (0, 1):
                    nc.sync.dma_start(
                        out=v_out[h * 4 : (h + 1) * 4, :],
                        in_=v_flat[:, h : h + 1],
                    )

    # The SPMD/axon path serializes nc.m directly without running Bacc's
    # pass pipeline; this walrus build allows at most one sync wait per
    # instruction, so split multi-waits into EventSemaphore instructions.
    _bass_rust.generate_event_semaphores(nc)
    return nc


_NC_CACHE = None


def _get_nc():
    global _NC_CACHE
    if _NC_CACHE is None:
        _NC_CACHE = _build_nc()
    return _NC_CACHE


def kernel(input_data, W, _trace=False, _tmpdir=None):
    input_data = np.ascontiguousarray(np.asarray(input_data, dtype=np.float32))
    W = np.ascontiguousarray(np.asarray(W, dtype=np.float32))
    assert input_data.shape == (I_CAPS, IN_D, 1)
    assert W.shape == (N_CAPS, I_CAPS, CAP_D, IN_D)

    x2 = np.ascontiguousarray(input_data[:, :, 0])
    eye = np.eye(128, dtype=np.float32)
    p_grp = np.arange(128) // 32  # partition -> local capsule index
    e_h = []
    for h in (0, 1):
        e = np.zeros((N_LOC, 128), dtype=np.float32)
        for n_ in range(N_LOC):
            e[n_] = (p_grp == n_ - 4 * h).astype(np.float32)
        e_h.append(e)
    m4_np = (p_grp[:, None] == np.arange(4)[None, :]).astype(np.float32)
    consts = {
        "ident": eye,
        "ident16": eye.astype(np.float16),
        "e0": e_h[0],
        "e1": e_h[1],
        "m8_0": np.ascontiguousarray(e_h[0].T),
        "m8_1": np.ascontiguousarray(e_h[1].T),
        "m4": m4_np,
    }
    in_maps = [
        {
            "w": np.ascontiguousarray(W[c * N_LOC : (c + 1) * N_LOC]),
            "x": x2,
            **consts,
        }
        for c in range(N_CORES)
    ]
    nc = _get_nc()
    out = run_bass_kernel_spmd(
        nc,
        in_maps,
        core_ids=list(range(N_CORES)),
        trace=_trace,
        tmpdir=_tmpdir,
    )
    res = out.results if hasattr(out, "results") else out
    v = np.concatenate([res[c]["v_out"] for c in range(N_CORES)], axis=0)
    if _trace:
        kernel.last_exec_time_ns = out.exec_time_ns
        kernel.last_results = out
    return v[..., None].astype(np.float32)


if __name__ == "__main__":
    rng = np.random.default_rng(0)
    inp = {
        "input_data": rng.standard_normal((I_CAPS, IN_D, 1)).astype(np.float32),
        "W": (rng.standard_normal((N_CAPS, I_CAPS, CAP_D, IN_D)) * 0.05).astype(
            np.float32
        ),
    }
    v = kernel(**inp)
    print("kernel output:", v.shape, v.dtype, "norm", np.linalg.norm(v))
